# revision 45
# baseline (speedup 1.0000x reference)
"""Trainium2 Bass kernel for nn_AttentionLayer (B=4, S=2048, D=1024, H=16).

Self-contained: builds and compiles an SPMD Bass/Tile program once, then
runs it across 8 NeuronCores via run_bass_kernel_spmd.

Sharding (no collectives): core c handles batch b = c // 2 and query-token
half c % 2 (1024 query tokens). Each core receives pre-transposed
activations (x^T slices) plus weights, computes its [1024, 1024] slice of
the final layernorm output in fp32, and the host reassembles.

Numerics: the attention path (K/V projections, exp weights, attn@V, FC)
runs on fp8-e4m3 operands — its contribution to the output is ~70x
smaller than the residual, so fp8 error is attenuated well below the
tolerance. The residual path (Q projection) stays bf16. Weights arrive
pre-scaled by 64 (fp8 dynamic range); the evacuations and the layernorm
scale-invariance absorb the factors exactly (powers of 2).

Scheduling: the PE clock gate (HAM) halves the TensorE clock whenever
the engine sees idle windows, so the kernel keeps TensorE saturated:
K (e>=1) / V projections plus the next pair's Q^T and residual
projections are emitted as single-matmul thunks interleaved into the
exp-paced attention loop, and attn@V runs as plain per-kb matmuls
(fp8 operands at bf16 rate) rather than DoubleRow so the queue never
drains. attn@V emission is gated on V-projection progress via a
backlog; per-pair score emission is hard-gated on K-projection
progress.

Per-core pipeline:
- Eager kpT[e=0] (plain fp8), chunked loads; everything else projects
  inside attention.
- Attention per head-pair: scores^T = Kh(fp8) @ Qh^T(bf16) into
  double-buffered PSUM, exp on ScalarE (bf16 out), attn@V with a
  per-head ones column (x8 scale) producing the softmax denominator
  row.
- Softmax denominators: DRAM roundtrip broadcast + fast reciprocal; the
  normalize multiply writes fp8 ctx (x64) for the fp8 FC.
- FC (plain fp8, bias via a K=1 ones matmul) + residual (x4096, exact
  power-of-2, absorbed by layernorm scale-invariance) + layernorm per
  128-token block.
"""

import numpy as np
import ml_dtypes


from collections import deque
from contextlib import ExitStack

import concourse.bass as bass
import concourse.tile as tile
import concourse.mybir as mybir
from concourse import bacc

F32 = mybir.dt.float32
BF16 = mybir.dt.bfloat16
F8 = mybir.dt.float8e4
AF = mybir.ActivationFunctionType
ALU = mybir.AluOpType
DR = mybir.MatmulPerfMode.DoubleRow


def bcast_ap(ap: bass.AP, parts: int) -> bass.AP:
    """Partition-broadcast a [1, N]-shaped DRAM AP to [parts, N]."""
    return bass.AP(tensor=ap.tensor, offset=ap.offset,
                   ap=[[0, parts]] + list(ap.ap[-1:]))


def nsplits(total, cap=512):
    return [(i, min(cap, total - i)) for i in range(0, total, cap)]


def build(T=1024, S=2048, D=1024, H=16, DK=64, n_cores=8, eps=1e-5,
          trn_type="TRN2", debug=False):
    assert DK == 64 and H % 2 == 0 and D == H * DK
    DB = D // 128     # 128-row contraction blocks over d
    D2 = DB // 2      # DoubleRow-paired contraction blocks
    EB = D // 128     # projection output blocks; == H//2
    TB = T // 128
    SB = S // 128
    S2 = SB // 2      # 256-token kv blocks
    PAIRS = H // 2
    VW = 65           # per-head vp stripe: 64 v columns + 1 ones column

    nc = bacc.Bacc(trn_type, target_bir_lowering=False, debug=False,
                   num_devices=n_cores)

    qT = nc.dram_tensor("qT", [D, T], BF16, kind="ExternalInput").ap()
    kT = nc.dram_tensor("kT", [D, S], F8, kind="ExternalInput").ap()
    vT = nc.dram_tensor("vT", [D, S], F8, kind="ExternalInput").ap()
    Wq = nc.dram_tensor("Wq", [D, D], BF16, kind="ExternalInput").ap()
    Wk = nc.dram_tensor("Wk", [D, D], F8, kind="ExternalInput").ap()   # x64
    Wv = nc.dram_tensor("Wv", [D, D], F8, kind="ExternalInput").ap()   # x64
    Wfc = nc.dram_tensor("Wfc", [D, D], F8, kind="ExternalInput").ap()  # x64
    bq = nc.dram_tensor("bq", [D], F32, kind="ExternalInput").ap()
    bk = nc.dram_tensor("bk", [D], F32, kind="ExternalInput").ap()
    bv = nc.dram_tensor("bv", [D], F32, kind="ExternalInput").ap()
    bfc = nc.dram_tensor("bfc", [D], F32, kind="ExternalInput").ap()
    gamma = nc.dram_tensor("gamma", [D], F32, kind="ExternalInput").ap()
    beta = nc.dram_tensor("beta", [D], F32, kind="ExternalInput").ap()
    out = nc.dram_tensor("out", [T, D], F32, kind="ExternalOutput").ap()
    den_dram = nc.dram_tensor("den_scratch", [H, T], F32).ap()
    dbg = {}
    if debug:
        for nm, shape in [("dbg_qpT", [128, T]), ("dbg_kpT", [128, S]),
                          ("dbg_vp", [128, 2 * H * VW]),
                          ("dbg_ctx8", [128, 2 * T]), ("dbg_at", [128, 2 * T]),
                          ("dbg_qp", [128, D])]:
            dt = F8 if nm in ("dbg_vp", "dbg_ctx8", "dbg_at",
                              "dbg_kpT") else BF16
            dbg[nm] = nc.dram_tensor(nm, shape, dt,
                                     kind="ExternalOutput").ap()

    def pair_ap(w, d2):
        """[128, 2, N] paired view of rows d2*256:(d2+1)*256 of a DRAM
        weight: partition p, slot i <- row d2*256 + i*128 + p."""
        return w[d2 * 256:(d2 + 1) * 256, :].rearrange(
            "(i p) n -> p i n", p=128)

    with tile.TileContext(nc) as tc, ExitStack() as ctx:
        pconst = ctx.enter_context(tc.tile_pool(name="const", bufs=1))
        ppers = ctx.enter_context(tc.tile_pool(name="persist", bufs=1))

        # ---- tiny constants (gpsimd DMA queue) ------------------------
        bqT = pconst.tile([128, EB], F32, tag="bqT", name="bqT")
        nc.gpsimd.dma_start(out=bqT, in_=bq.rearrange("(e p) -> p e", p=128))
        bkT = pconst.tile([128, EB], F32, tag="bkT", name="bkT")
        nc.gpsimd.dma_start(out=bkT, in_=bk.rearrange("(e p) -> p e", p=128))
        eps_t = pconst.tile([128, 1], F32, tag="eps", name="eps")
        nc.vector.memset(eps_t, eps)
        ones_1 = pconst.tile([1, 128], BF16, tag="ones1", name="ones1")
        nc.vector.memset(ones_1, 1.0)
        # bias rows for K=1 matmuls: bv*64 and (bq+bfc)*4096, in bf16
        bv_row = pconst.tile([1, D], F32, tag="bv_row", name="bv_row")
        nc.gpsimd.dma_start(out=bv_row, in_=bv.rearrange("(o n) -> o n", o=1))
        bv64_n = pconst.tile([1, D], BF16, tag="bv64_n", name="bv64_n")
        nc.vector.tensor_scalar(out=bv64_n, in0=bv_row, scalar1=64.0,
                                scalar2=None, op0=ALU.mult)
        bq_row = pconst.tile([1, D], F32, tag="bq_row", name="bq_row")
        nc.gpsimd.dma_start(out=bq_row, in_=bq.rearrange("(o n) -> o n", o=1))
        bfc_row = pconst.tile([1, D], F32, tag="bfc_row", name="bfc_row")
        nc.gpsimd.dma_start(out=bfc_row, in_=bfc.rearrange("(o n) -> o n", o=1))
        bqfc_row = pconst.tile([1, D], F32, tag="bqfc_row", name="bqfc_row")
        nc.vector.tensor_add(out=bqfc_row, in0=bq_row, in1=bfc_row)
        bqfc4096_n = pconst.tile([1, D], BF16, tag="bqfc4096", name="bqfc4096")
        nc.vector.tensor_scalar(out=bqfc4096_n, in0=bqfc_row, scalar1=4096.0,
                                scalar2=None, op0=ALU.mult)
        gamma_bc = pconst.tile([128, D], F32, tag="gamma_bc", name="gamma_bc")
        nc.gpsimd.dma_start(out=gamma_bc, in_=bcast_ap(gamma, 128))
        beta_bc = pconst.tile([128, D], F32, tag="beta_bc", name="beta_bc")
        nc.gpsimd.dma_start(out=beta_bc, in_=bcast_ap(beta, 128))

        # ---- persistent SBUF ------------------------------------------
        kpT_sb = [ppers.tile([128, S], F8, tag=f"kpT{e}", name=f"kpT{e}")
                  for e in range(EB)]
        vp2_sb = [ppers.tile([128, 2 * H * VW], F8, tag=f"vp{s2}",
                             name=f"vp{s2}") for s2 in range(S2)]
        ctx8_sb = [ppers.tile([128, 2 * T], F8, tag=f"ctx8_{jp}",
                              name=f"ctx8_{jp}") for jp in range(PAIRS // 2)]
        qp_sb = [ppers.tile([128, D], BF16, tag=f"qp{t}", name=f"qp{t}")
                 for t in range(TB)]

        pqx = ctx.enter_context(tc.tile_pool(name="qx", bufs=1))
        pwq = ctx.enter_context(tc.tile_pool(name="wq", bufs=1))
        qx_sb = [pqx.tile([128, T], BF16, tag=f"qx{d}", name=f"qx{d}")
                 for d in range(DB)]
        wq_sb = [pwq.tile([128, D], BF16, tag=f"wq{d}", name=f"wq{d}")
                 for d in range(DB)]
        # fp8 operand pools that live through the attention phase
        pwv = ctx.enter_context(tc.tile_pool(name="wv", bufs=1))
        pvx = ctx.enter_context(tc.tile_pool(name="vx", bufs=2 * D2))
        pwfc = ctx.enter_context(tc.tile_pool(name="wfc", bufs=1))
        wv8_sb = [pwv.tile([128, 2 * D], F8, tag=f"wv{d2}", name=f"wv{d2}")
                  for d2 in range(D2)]
        wfc8_sb = [pwfc.tile([128, 2 * D], F8, tag=f"wfc{jp}",
                             name=f"wfc{jp}") for jp in range(D2)]

        CK = 512
        NCK = len(nsplits(S, CK))
        SPC = CK // 128
        c0s = nsplits(S, CK)

        # K operands stay resident: e=0 projects eagerly, e=1..7 project
        # as thunks inside the attention loop (plain-fp8 matmuls — bf16
        # rate — so TensorE stays saturated and the HAM clock stays warm).
        pwk = ctx.enter_context(tc.tile_pool(name="wk", bufs=1))
        pkx = ctx.enter_context(tc.tile_pool(name="kx", bufs=1))
        wk8_sb = [pwk.tile([128, 2 * D], F8, tag=f"wk{d2}",
                           name=f"wk{d2}") for d2 in range(D2)]
        kx_t = [[pkx.tile([128, 2 * CK], F8, tag=f"kx{d2}_{ci}",
                          name=f"kx{d2}_{ci}") for d2 in range(D2)]
                for ci in range(NCK)]

        # ============ loads ============================================
        if True:
            for d2 in range(D2):
                nc.sync.dma_start(
                    out=wk8_sb[d2].rearrange("p (i n) -> p i n", i=2),
                    in_=pair_ap(Wk, d2))
                c0, cn = c0s[0]
                nc.sync.dma_start(
                    out=kx_t[0][d2].rearrange("p (i c) -> p i c", i=2),
                    in_=kT[d2 * 256:(d2 + 1) * 256, c0:c0 + cn].rearrange(
                        "(i p) c -> p i c", p=128))
            for d in range(DB):
                nc.sync.dma_start(out=qx_sb[d], in_=qT[d * 128:(d + 1) * 128, :])
                nc.sync.dma_start(out=wq_sb[d], in_=Wq[d * 128:(d + 1) * 128, :])
            for ci, (c0, cn) in list(enumerate(c0s))[1:]:
                for d2 in range(D2):
                    nc.sync.dma_start(
                        out=kx_t[ci][d2].rearrange("p (i c) -> p i c", i=2),
                        in_=kT[d2 * 256:(d2 + 1) * 256, c0:c0 + cn].rearrange(
                            "(i p) c -> p i c", p=128))
            for d2 in range(D2):
                nc.sync.dma_start(
                    out=wv8_sb[d2].rearrange("p (i n) -> p i n", i=2),
                    in_=pair_ap(Wv, d2))
            vx_t = []
            for ci, (c0, cn) in enumerate(c0s):
                tiles = [pvx.tile([128, 2 * CK], F8, tag="vx",
                                  name=f"vx{d2}_{ci}") for d2 in range(D2)]
                for d2 in range(D2):
                    nc.sync.dma_start(
                        out=tiles[d2].rearrange("p (i c) -> p i c", i=2),
                        in_=vT[d2 * 256:(d2 + 1) * 256, c0:c0 + cn].rearrange(
                            "(i p) c -> p i c", p=128))
                vx_t.append(tiles)
            for jp in range(D2):
                nc.gpsimd.dma_start(
                    out=wfc8_sb[jp].rearrange("p (i n) -> p i n", i=2),
                    in_=pair_ap(Wfc, jp))


        # ================= attention ====================================
        with tc.tile_pool(name="scps", bufs=2, space="PSUM") as psc, \
             tc.tile_pool(name="cxps", bufs=1, space="PSUM") as pcx, \
             tc.tile_pool(name="pjps", bufs=2, space="PSUM") as ppj, \
             tc.tile_pool(name="qpT", bufs=2) as pqpt, \
             tc.tile_pool(name="attn", bufs=8) as patn, \
             tc.tile_pool(name="norm", bufs=1) as pnm, \
             tc.tile_pool(name="cpair", bufs=1) as pcp, \
             tc.tile_pool(name="ctmp", bufs=2) as ptmp:
            qpT_tiles = {}
            vp_ready = [0]
            kp_ready = [0] * EB  # chunks of kpT[e] projected

            # ---- K-projection thunk groups (plain fp8, per (e, ci)) ---
            def make_k_groups():
                groups = []

                def k_mm(e, ci, cn, d2, i, state):
                    def f():
                        if 'ps' not in state:
                            state['ps'] = ppj.tile([128, CK], F32, tag="pj",
                                                   name="pjk")
                        wv_ = wk8_sb[d2].rearrange("p (i n) -> p i n", i=2)
                        kxv = kx_t[ci][d2].rearrange("p (i c) -> p i c", i=2)
                        nc.tensor.matmul(
                            state['ps'][:, 0:cn],
                            lhsT=wv_[:, i, e * 128:(e + 1) * 128],
                            rhs=kxv[:, i, 0:cn],
                            start=(d2 == 0 and i == 0),
                            stop=(d2 == D2 - 1 and i == 1))
                    return f

                def k_evac(e, ci, c0, cn, state):
                    def f():
                        nc.vector.tensor_scalar(
                            out=kpT_sb[e][:, c0:c0 + cn],
                            in0=state['ps'][:, 0:cn],
                            scalar1=1.0 / 64, scalar2=bkT[:, e:e + 1],
                            op0=ALU.mult, op1=ALU.add)
                        kp_ready[e] += 1
                    return f

                for e in range(EB):
                    for ci, (c0, cn) in enumerate(c0s):
                        state = {}
                        g = [k_mm(e, ci, cn, d2, i, state)
                             for d2 in range(D2) for i in range(2)]
                        g.append(k_evac(e, ci, c0, cn, state))
                        groups.append(g)
                return groups

            # ---- V-projection thunks (fp8 DoubleRow, n-half groups) ---
            def make_v_thunks():
                thunks = []

                def v_mm(s, half, n0, nn, d2, state):
                    def f():
                        if half not in state:
                            state[half] = ppj.tile([128, 512], F32, tag="pj",
                                                   name="pjv")
                            nc.tensor.matmul(
                                state[half][:, 0:nn], lhsT=ones_1,
                                rhs=bv64_n[0:1, n0:n0 + nn],
                                start=True, stop=False)
                        vxv = vx_t[s // SPC][d2].rearrange(
                            "p (i c) -> p i c", i=2)
                        wvv = wv8_sb[d2].rearrange("p (i n) -> p i n", i=2)
                        nc.tensor.matmul(
                            state[half][:, 0:nn],
                            lhsT=vxv[:, :, (s % SPC) * 128:(s % SPC + 1) * 128],
                            rhs=wvv[:, :, n0:n0 + nn],
                            start=False, stop=(d2 == D2 - 1),
                            perf_mode=DR)
                    return f

                def v_evac(s, half, n0, nn, state):
                    def f():
                        # features n0:n0+nn == heads n0//64 : (n0+nn)//64
                        h0, h1 = n0 // DK, (n0 + nn) // DK
                        vr = vp2_sb[s // 2].rearrange(
                            "p (i h c) -> p i h c", i=2, c=VW)
                        nc.vector.tensor_scalar(
                            out=vr[:, s % 2, h0:h1, 0:DK],
                            in0=state[half].rearrange(
                                "p (h c) -> p h c", c=DK)[:, 0:(h1 - h0), :],
                            scalar1=0.125, scalar2=None, op0=ALU.mult)
                        nc.vector.memset(vr[:, s % 2, h0:h1, DK:VW], 8.0)
                        if half == 1 and s % 2 == 1:
                            vp_ready[0] += 1
                    return f

                for s in range(SB):
                    state = {}
                    for half, (n0, nn) in enumerate(nsplits(D)):
                        g = [v_mm(s, half, n0, nn, d2, state)
                             for d2 in range(D2)]
                        g.append(v_evac(s, half, n0, nn, state))
                        thunks.append(g)
                return thunks

            # ---- Q^T projection (bf16) + residual qp (x4096) thunks ---
            def make_q_thunks(jj):
                state = {}
                thunks = []

                def q_mm(half, n0, nn, d):
                    def f():
                        if half not in state:
                            state[half] = ppj.tile([128, 512], F32, tag="pj",
                                                   name="pjq")
                        nc.tensor.matmul(
                            state[half][:, 0:nn],
                            lhsT=wq_sb[d][:, jj * 128:(jj + 1) * 128],
                            rhs=qx_sb[d][:, n0:n0 + nn],
                            start=(d == 0), stop=(d == DB - 1))
                    return f

                def q_evac(half, n0, nn):
                    def f():
                        if 'qt' not in state:
                            state['qt'] = pqpt.tile([128, T], BF16,
                                                    tag="qpT_t", name="qpT_t")
                            qpT_tiles[jj] = state['qt']
                        nc.vector.tensor_scalar(
                            out=state['qt'][:, n0:n0 + nn],
                            in0=state[half][:, 0:nn],
                            scalar1=bqT[:, jj:jj + 1], scalar2=None,
                            op0=ALU.add)
                    return f

                for half, (n0, nn) in enumerate(nsplits(T)):
                    for d in range(DB):
                        thunks.append(q_mm(half, n0, nn, d))
                    thunks.append(q_evac(half, n0, nn))
                return thunks

            def make_qp_thunks(t):
                """Residual qp[t] = 4096*(q @ Wq + bq) per 128-token block,
                bf16; the 4096 matches the fp8 FC scale, layernorm is
                scale-invariant."""
                state = {}
                thunks = []

                def qp_mm(half, n0, nn, d):
                    def f():
                        if half not in state:
                            state[half] = ppj.tile([128, 512], F32, tag="pj",
                                                   name="pjr")
                        nc.tensor.matmul(
                            state[half][:, 0:nn],
                            lhsT=qx_sb[d][:, t * 128:(t + 1) * 128],
                            rhs=wq_sb[d][:, n0:n0 + nn],
                            start=(d == 0), stop=(d == DB - 1))
                    return f

                def qp_evac(half, n0, nn):
                    def f():
                        nc.vector.tensor_scalar(
                            out=qp_sb[t][:, n0:n0 + nn],
                            in0=state[half][:, 0:nn],
                            scalar1=4096.0, scalar2=None, op0=ALU.mult)
                    return f

                for half, (n0, nn) in enumerate(nsplits(D)):
                    for d in range(DB):
                        thunks.append(qp_mm(half, n0, nn, d))
                    thunks.append(qp_evac(half, n0, nn))
                return thunks

            kg = make_k_groups()
            # eager: kpT[0] chunk 0, then pair 0's Q^T (so the two DMA
            # stalls overlap), then kpT[0] chunks 1..3
            for th in kg[0]:
                th()
            for th in make_q_thunks(0):
                th()
            for g in kg[1:NCK]:
                for th in g:
                    th()
            # mixed background queue: 1 K-group : 2 V-half-groups, so the
            # per-pair kpT gate leads the per-kb vp gate comfortably
            vg = make_v_thunks()
            vq = deque()
            ki, vi = NCK, 0
            while ki < len(kg) or vi < len(vg):
                if ki < len(kg):
                    vq.extend(kg[ki])
                    ki += 1
                for _ in range(2):
                    if vi < len(vg):
                        vq.extend(vg[vi])
                        vi += 1

            av_jobs = deque()

            def drain_av():
                while av_jobs and 2 * vp_ready[0] > av_jobs[0][1]:
                    h_, kb_, cx_, at_ = av_jobs.popleft()
                    vr = vp2_sb[kb_ // 2].rearrange("p (i h c) -> p i h c",
                                                    i=2, c=VW)
                    for n0, nn in nsplits(T):
                        nc.tensor.matmul(
                            cx_[:, n0:n0 + nn],
                            lhsT=vr[:, kb_ % 2, h_, :],
                            rhs=at_[:, n0:n0 + nn],
                            start=(kb_ == 0), stop=(kb_ == SB - 1))

            for j in range(PAIRS):
                # hard gate: kpT[j] must be fully projected before scores
                while kp_ready[j] < NCK:
                    vq.popleft()()
                qpT_t = qpT_tiles.pop(j)
                if debug and j == 0:
                    nc.sync.dma_start(out=dbg["dbg_qpT"], in_=qpT_t)
                pending = list(make_q_thunks(j + 1)) if j + 1 < PAIRS else []
                if j < PAIRS - 1:
                    pending += make_qp_thunks(j)
                    if j == PAIRS - 2:
                        pending += make_qp_thunks(PAIRS - 1)
                dbc = pnm.tile([128, T], F32, tag="dbc", name="dbc")
                for hh in range(2):
                    h = 2 * j + hh
                    pr = slice(hh * 64, hh * 64 + 64)
                    cx = pcx.tile([VW, T], F32, tag="cx", name="cx")
                    for kb in range(SB):
                        at = patn.tile([128, T], BF16, tag="at", name="at")
                        sc = psc.tile([128, T], F32, tag="sc", name="sc")
                        for n0, nn in nsplits(T):
                            nc.tensor.matmul(
                                sc[:, n0:n0 + nn],
                                lhsT=kpT_sb[j][pr, kb * 128:(kb + 1) * 128],
                                rhs=qpT_t[pr, n0:n0 + nn],
                                start=True, stop=True)
                        nc.scalar.activation(out=at, in_=sc,
                                             func=AF.Exp, scale=0.125)
                        for _ in range(6 if j < 2 else 3):
                            if vq:
                                vq.popleft()()
                            elif pending:
                                pending.pop(0)()
                            else:
                                break
                        av_jobs.append((h, kb, cx, at))
                        drain_av()
                    # force-finish this head's attn@V before evacuating
                    while av_jobs:
                        while vq and 2 * vp_ready[0] <= av_jobs[0][1]:
                            vq.popleft()()
                        drain_av()
                    # evacuate unnormalized ctx (bf16) + denominator/64
                    den = pnm.tile([VW, T], F32, tag="den", name="den")
                    nc.vector.tensor_scalar(
                        out=den[DK:VW, :], in0=cx[DK:VW, :],
                        scalar1=1.0 / 64, scalar2=None, op0=ALU.mult)
                    nc.gpsimd.dma_start(out=den_dram[h, :], in_=den[DK:VW, :])
                    nc.gpsimd.dma_start(
                        out=dbc[hh * 64:(hh + 1) * 64, :],
                        in_=bcast_ap(den_dram[h:h + 1, :], 64))
                    if hh == 0:
                        cpair = pcp.tile([128, T], BF16, tag="cpair",
                                         name="cpair")
                        nc.vector.tensor_copy(out=cpair[0:64, :],
                                              in_=cx[0:DK, :])
                    else:
                        tmp = ptmp.tile([64, T], BF16, tag="ctmp", name="ctmp")
                        nc.vector.tensor_copy(out=tmp, in_=cx[0:DK, :])
                        nc.sync.dma_start(out=cpair[64:128, :], in_=tmp)
                while pending:
                    pending.pop(0)()
                # normalize: ctx8[jp][:, j%2, :] = cpair * (64/den)  in fp8
                rbc = pnm.tile([128, T], F32, tag="rbc", name="rbc")
                nc.vector.reciprocal_approx_fast(out=rbc, in_=dbc)
                c8 = ctx8_sb[j // 2].rearrange("p (i n) -> p i n", i=2)
                nc.vector.tensor_mul(out=c8[:, j % 2, :], in0=cpair, in1=rbc)

        # ========= FC (fp8 DoubleRow) + residual + layernorm ===========
        with tc.tile_pool(name="fcps", bufs=3, space="PSUM") as pfc, \
             tc.tile_pool(name="xln", bufs=3) as px, \
             tc.tile_pool(name="stat", bufs=6) as pst:
            for t in range(TB):
                fc = pfc.tile([128, D], F32, tag="fc", name="fc")
                for n0, nn in nsplits(D):
                    nc.tensor.matmul(
                        fc[:, n0:n0 + nn], lhsT=ones_1,
                        rhs=bqfc4096_n[0:1, n0:n0 + nn],
                        start=True, stop=False)
                    for jp in range(D2):
                        c8 = ctx8_sb[jp].rearrange("p (i n) -> p i n", i=2)
                        w8 = wfc8_sb[jp].rearrange("p (i n) -> p i n", i=2)
                        for i in range(2):
                            nc.tensor.matmul(
                                fc[:, n0:n0 + nn],
                                lhsT=c8[:, i, t * 128:(t + 1) * 128],
                                rhs=w8[:, i, n0:n0 + nn],
                                start=False,
                                stop=(jp == D2 - 1 and i == 1))
                x = px.tile([128, D], F32, tag="x", name="x")
                nc.vector.tensor_add(out=x, in0=fc, in1=qp_sb[t])
                ngr = max(D // 512, 1)
                gsz = min(D, 512)
                stats = pst.tile([128, ngr, 6], F32, tag="stats", name="stats")
                for g in range(ngr):
                    nc.vector.bn_stats(out=stats[:, g, :],
                                       in_=x[:, g * gsz:(g + 1) * gsz])
                mv = pst.tile([128, 2], F32, tag="mv", name="mv")
                nc.vector.bn_aggr(out=mv, in_=stats)
                rstd = pst.tile([128, 1], F32, tag="rstd", name="rstd")
                nc.scalar.activation(out=rstd, in_=mv[:, 1:2], func=AF.Sqrt,
                                     bias=eps_t, scale=1.0)
                nc.vector.reciprocal(out=rstd, in_=rstd)
                xn = px.tile([128, D], F32, tag="xn", name="xn")
                nc.vector.tensor_scalar(out=xn, in0=x, scalar1=mv[:, 0:1],
                                        scalar2=rstd, op0=ALU.subtract,
                                        op1=ALU.mult)
                nc.vector.tensor_mul(out=xn, in0=xn, in1=gamma_bc)
                nc.gpsimd.tensor_add(out=xn, in0=xn, in1=beta_bc)
                nc.sync.dma_start(out=out[t * 128:(t + 1) * 128, :], in_=xn)

            if debug:
                nc.sync.dma_start(out=dbg["dbg_kpT"], in_=kpT_sb[0])
                nc.sync.dma_start(out=dbg["dbg_vp"], in_=vp2_sb[0])
                nc.sync.dma_start(out=dbg["dbg_ctx8"], in_=ctx8_sb[0])
                nc.sync.dma_start(out=dbg["dbg_qp"], in_=qp_sb[0])

    nc.compile()
    return nc


_B, _S, _D, _H, _DK = 4, 2048, 1024, 16, 64
_T = _S // 2
_NCORES = 8
_BF = ml_dtypes.bfloat16
_F8 = ml_dtypes.float8_e4m3

_nc_cache = [None]


def _get_nc():
    if _nc_cache[0] is None:
        _nc_cache[0] = build(T=_T, S=_S, D=_D, H=_H, DK=_DK, n_cores=_NCORES)
    return _nc_cache[0]


def _f8(a):
    return np.clip(a, -240.0, 240.0).astype(_F8)


def _make_in_maps(inputs):
    q = np.asarray(inputs["q"], np.float32)
    k = np.asarray(inputs["k"], np.float32)
    v = np.asarray(inputs["v"], np.float32)
    Wq = np.asarray(inputs["Wq"], np.float32).astype(_BF)
    Wk = _f8(np.asarray(inputs["Wk"], np.float32) * 64.0)
    Wv = _f8(np.asarray(inputs["Wv"], np.float32) * 64.0)
    Wfc = _f8(np.asarray(inputs["Wfc"], np.float32) * 64.0)
    fp = {n: np.asarray(inputs[n], np.float32)
          for n in ("bq", "bk", "bv", "bfc", "gamma", "beta")}

    in_maps = []
    for c in range(_NCORES):
        b, half = divmod(c, 2)
        t0 = half * _T
        in_maps.append({
            "qT": np.ascontiguousarray(q[b, t0:t0 + _T].T).astype(_BF),
            "kT": _f8(np.ascontiguousarray(k[b].T)),
            "vT": _f8(np.ascontiguousarray(v[b].T)),
            "Wq": Wq, "Wk": Wk, "Wv": Wv, "Wfc": Wfc, **fp,
        })
    return in_maps


def _execute(inputs, trace=False, tmpdir=None):
    from concourse.bass_utils import run_bass_kernel_spmd

    nc = _get_nc()
    in_maps = _make_in_maps(inputs)
    res = run_bass_kernel_spmd(nc, in_maps, core_ids=list(range(_NCORES)),
                               trace=trace, tmpdir=tmpdir)
    out = np.empty((_B, _S, _D), np.float32)
    for c in range(_NCORES):
        b, half = divmod(c, 2)
        out[b, half * _T:(half + 1) * _T] = res.results[c]["out"]
    return out, res.exec_time_ns


def kernel(**inputs) -> np.ndarray:
    out, _ = _execute(inputs, trace=False)
    return out


# revision 46
# speedup vs baseline: 1.2453x; 1.2453x over previous
"""Trainium2 Bass kernel for nn_AttentionLayer (B=4, S=2048, D=1024, H=16).

Self-contained: builds and compiles an SPMD Bass/Tile program once, then
runs it across 8 NeuronCores via run_bass_kernel_spmd.

Sharding (no collectives): core c handles batch b = c // 2 and query-token
half c % 2 (1024 query tokens). Each core receives pre-transposed
activations (x^T slices) plus weights, computes its [1024, 1024] slice of
the final layernorm output in fp32, and the host reassembles.

Numerics: the attention path (K/V projections, exp weights, attn@V, FC)
runs on fp8-e4m3 operands — its contribution to the output is ~70x
smaller than the residual, so fp8 error is attenuated well below the
tolerance. The residual path (Q projection) stays bf16. Weights arrive
pre-scaled by 64 (fp8 dynamic range); the evacuations and the layernorm
scale-invariance absorb the factors exactly (powers of 2).

Scheduling: the PE clock gate (HAM) halves the TensorE clock whenever
the engine sees idle windows, so the kernel keeps TensorE saturated:
K (e>=1) / V projections plus the next pair's Q^T and residual
projections are emitted as single-matmul thunks interleaved into the
exp-paced attention loop, and attn@V runs as plain per-kb matmuls
(fp8 operands at bf16 rate) rather than DoubleRow so the queue never
drains. attn@V emission is gated on V-projection progress via a
backlog; per-pair score emission is hard-gated on K-projection
progress.

Per-core pipeline:
- Eager kpT[e=0] (plain fp8), chunked loads; everything else projects
  inside attention.
- Attention per head-pair: scores^T = Kh(fp8) @ Qh^T(bf16) into
  double-buffered PSUM, exp on ScalarE (bf16 out), attn@V with a
  per-head ones column (x8 scale) producing the softmax denominator
  row.
- Softmax denominators: DRAM roundtrip broadcast + fast reciprocal; the
  normalize multiply writes fp8 ctx (x64) for the fp8 FC.
- FC (plain fp8, bias via a K=1 ones matmul) + residual (x4096, exact
  power-of-2, absorbed by layernorm scale-invariance) + layernorm per
  128-token block.
"""

import numpy as np
import ml_dtypes


from collections import deque
from contextlib import ExitStack

import concourse.bass as bass
import concourse.tile as tile
import concourse.mybir as mybir
from concourse import bacc

F32 = mybir.dt.float32
BF16 = mybir.dt.bfloat16
F8 = mybir.dt.float8e4
AF = mybir.ActivationFunctionType
ALU = mybir.AluOpType
DR = mybir.MatmulPerfMode.DoubleRow


def bcast_ap(ap: bass.AP, parts: int) -> bass.AP:
    """Partition-broadcast a [1, N]-shaped DRAM AP to [parts, N]."""
    return bass.AP(tensor=ap.tensor, offset=ap.offset,
                   ap=[[0, parts]] + list(ap.ap[-1:]))


def nsplits(total, cap=512):
    return [(i, min(cap, total - i)) for i in range(0, total, cap)]


def build(T=1024, S=2048, D=1024, H=16, DK=64, n_cores=8, eps=1e-5,
          trn_type="TRN2", debug=False):
    assert DK == 64 and H % 2 == 0 and D == H * DK
    DB = D // 128     # 128-row contraction blocks over d
    D2 = DB // 2      # DoubleRow-paired contraction blocks
    EB = D // 128     # projection output blocks; == H//2
    TB = T // 128
    SB = S // 128
    S2 = SB // 2      # 256-token kv blocks
    PAIRS = H // 2
    VW = 65           # per-head vp stripe: 64 v columns + 1 ones column

    nc = bacc.Bacc(trn_type, target_bir_lowering=False, debug=False,
                   num_devices=n_cores)

    qT = nc.dram_tensor("qT", [D, T], BF16, kind="ExternalInput").ap()
    kT = nc.dram_tensor("kT", [D, S], F8, kind="ExternalInput").ap()
    vT = nc.dram_tensor("vT", [D, S], F8, kind="ExternalInput").ap()
    Wq = nc.dram_tensor("Wq", [D, D], BF16, kind="ExternalInput").ap()
    Wk = nc.dram_tensor("Wk", [D, D], F8, kind="ExternalInput").ap()   # x64
    Wv = nc.dram_tensor("Wv", [D, D], F8, kind="ExternalInput").ap()   # x64
    Wfc = nc.dram_tensor("Wfc", [D, D], F8, kind="ExternalInput").ap()  # x64
    bq = nc.dram_tensor("bq", [D], F32, kind="ExternalInput").ap()
    bk = nc.dram_tensor("bk", [D], F32, kind="ExternalInput").ap()
    bv = nc.dram_tensor("bv", [D], F32, kind="ExternalInput").ap()
    bfc = nc.dram_tensor("bfc", [D], F32, kind="ExternalInput").ap()
    gamma = nc.dram_tensor("gamma", [D], F32, kind="ExternalInput").ap()
    beta = nc.dram_tensor("beta", [D], F32, kind="ExternalInput").ap()
    out = nc.dram_tensor("out", [T, D], F32, kind="ExternalOutput").ap()
    den_dram = nc.dram_tensor("den_scratch", [H, T], F32).ap()
    dbg = {}
    if debug:
        for nm, shape in [("dbg_qpT", [128, T]), ("dbg_kpT", [128, S]),
                          ("dbg_vp", [128, 2 * H * VW]),
                          ("dbg_ctx8", [128, 2 * T]), ("dbg_at", [128, 2 * T]),
                          ("dbg_qp", [128, D])]:
            dt = F8 if nm in ("dbg_vp", "dbg_ctx8", "dbg_at",
                              "dbg_kpT") else BF16
            dbg[nm] = nc.dram_tensor(nm, shape, dt,
                                     kind="ExternalOutput").ap()

    def pair_ap(w, d2):
        """[128, 2, N] paired view of rows d2*256:(d2+1)*256 of a DRAM
        weight: partition p, slot i <- row d2*256 + i*128 + p."""
        return w[d2 * 256:(d2 + 1) * 256, :].rearrange(
            "(i p) n -> p i n", p=128)

    with tile.TileContext(nc) as tc, ExitStack() as ctx:
        pconst = ctx.enter_context(tc.tile_pool(name="const", bufs=1))
        ppers = ctx.enter_context(tc.tile_pool(name="persist", bufs=1))

        # ---- tiny constants (gpsimd DMA queue) ------------------------
        bqT = pconst.tile([128, EB], F32, tag="bqT", name="bqT")
        nc.gpsimd.dma_start(out=bqT, in_=bq.rearrange("(e p) -> p e", p=128))
        bkT = pconst.tile([128, EB], F32, tag="bkT", name="bkT")
        nc.gpsimd.dma_start(out=bkT, in_=bk.rearrange("(e p) -> p e", p=128))
        eps_t = pconst.tile([128, 1], F32, tag="eps", name="eps")
        nc.vector.memset(eps_t, eps)
        ones_1 = pconst.tile([1, 128], BF16, tag="ones1", name="ones1")
        nc.vector.memset(ones_1, 1.0)
        # bias rows for K=1 matmuls: bv*64 and (bq+bfc)*4096, in bf16
        bv_row = pconst.tile([1, D], F32, tag="bv_row", name="bv_row")
        nc.gpsimd.dma_start(out=bv_row, in_=bv.rearrange("(o n) -> o n", o=1))
        bv64_n = pconst.tile([1, D], BF16, tag="bv64_n", name="bv64_n")
        nc.vector.tensor_scalar(out=bv64_n, in0=bv_row, scalar1=64.0,
                                scalar2=None, op0=ALU.mult)
        bq_row = pconst.tile([1, D], F32, tag="bq_row", name="bq_row")
        nc.gpsimd.dma_start(out=bq_row, in_=bq.rearrange("(o n) -> o n", o=1))
        bfc_row = pconst.tile([1, D], F32, tag="bfc_row", name="bfc_row")
        nc.gpsimd.dma_start(out=bfc_row, in_=bfc.rearrange("(o n) -> o n", o=1))
        bqfc_row = pconst.tile([1, D], F32, tag="bqfc_row", name="bqfc_row")
        nc.vector.tensor_add(out=bqfc_row, in0=bq_row, in1=bfc_row)
        bqfc4096_n = pconst.tile([1, D], BF16, tag="bqfc4096", name="bqfc4096")
        nc.vector.tensor_scalar(out=bqfc4096_n, in0=bqfc_row, scalar1=4096.0,
                                scalar2=None, op0=ALU.mult)
        gamma_bc = pconst.tile([128, D], F32, tag="gamma_bc", name="gamma_bc")
        nc.gpsimd.dma_start(out=gamma_bc, in_=bcast_ap(gamma, 128))
        beta_bc = pconst.tile([128, D], F32, tag="beta_bc", name="beta_bc")
        nc.gpsimd.dma_start(out=beta_bc, in_=bcast_ap(beta, 128))

        # ---- persistent SBUF ------------------------------------------
        kpT_sb = [ppers.tile([128, S], F8, tag=f"kpT{e}", name=f"kpT{e}")
                  for e in range(EB)]
        vp2_sb = [ppers.tile([128, 2 * H * VW], F8, tag=f"vp{s2}",
                             name=f"vp{s2}") for s2 in range(S2)]
        ctx8_sb = [ppers.tile([128, 2 * T], F8, tag=f"ctx8_{jp}",
                              name=f"ctx8_{jp}") for jp in range(PAIRS // 2)]
        qp_sb = [ppers.tile([128, D], BF16, tag=f"qp{t}", name=f"qp{t}")
                 for t in range(TB)]

        pqx = ctx.enter_context(tc.tile_pool(name="qx", bufs=1))
        pwq = ctx.enter_context(tc.tile_pool(name="wq", bufs=1))
        qx_sb = [pqx.tile([128, T], BF16, tag=f"qx{d}", name=f"qx{d}")
                 for d in range(DB)]
        wq_sb = [pwq.tile([128, D], BF16, tag=f"wq{d}", name=f"wq{d}")
                 for d in range(DB)]
        # fp8 operand pools that live through the attention phase
        pwv = ctx.enter_context(tc.tile_pool(name="wv", bufs=1))
        pvx = ctx.enter_context(tc.tile_pool(name="vx", bufs=2 * D2))
        pwfc = ctx.enter_context(tc.tile_pool(name="wfc", bufs=1))
        wv8_sb = [pwv.tile([128, 2 * D], F8, tag=f"wv{d2}", name=f"wv{d2}")
                  for d2 in range(D2)]
        wfc8_sb = [pwfc.tile([128, 2 * D], F8, tag=f"wfc{jp}",
                             name=f"wfc{jp}") for jp in range(D2)]

        CK = 512
        NCK = len(nsplits(S, CK))
        SPC = CK // 128
        c0s = nsplits(S, CK)

        # K operands stay resident: e=0 projects eagerly, e=1..7 project
        # as thunks inside the attention loop (plain-fp8 matmuls — bf16
        # rate — so TensorE stays saturated and the HAM clock stays warm).
        pwk = ctx.enter_context(tc.tile_pool(name="wk", bufs=1))
        pkx = ctx.enter_context(tc.tile_pool(name="kx", bufs=1))
        wk8_sb = [pwk.tile([128, 2 * D], F8, tag=f"wk{d2}",
                           name=f"wk{d2}") for d2 in range(D2)]
        kx_t = [[pkx.tile([128, 2 * CK], F8, tag=f"kx{d2}_{ci}",
                          name=f"kx{d2}_{ci}") for d2 in range(D2)]
                for ci in range(NCK)]

        # ============ loads ============================================
        if True:
            for d2 in range(D2):
                nc.sync.dma_start(
                    out=wk8_sb[d2].rearrange("p (i n) -> p i n", i=2),
                    in_=pair_ap(Wk, d2))
                c0, cn = c0s[0]
                nc.sync.dma_start(
                    out=kx_t[0][d2].rearrange("p (i c) -> p i c", i=2),
                    in_=kT[d2 * 256:(d2 + 1) * 256, c0:c0 + cn].rearrange(
                        "(i p) c -> p i c", p=128))
            for d in range(DB):
                nc.sync.dma_start(out=qx_sb[d], in_=qT[d * 128:(d + 1) * 128, :])
                nc.sync.dma_start(out=wq_sb[d], in_=Wq[d * 128:(d + 1) * 128, :])
            for ci, (c0, cn) in list(enumerate(c0s))[1:]:
                for d2 in range(D2):
                    nc.sync.dma_start(
                        out=kx_t[ci][d2].rearrange("p (i c) -> p i c", i=2),
                        in_=kT[d2 * 256:(d2 + 1) * 256, c0:c0 + cn].rearrange(
                            "(i p) c -> p i c", p=128))
            for d2 in range(D2):
                nc.sync.dma_start(
                    out=wv8_sb[d2].rearrange("p (i n) -> p i n", i=2),
                    in_=pair_ap(Wv, d2))
            vx_t = []
            for ci, (c0, cn) in enumerate(c0s):
                tiles = [pvx.tile([128, 2 * CK], F8, tag="vx",
                                  name=f"vx{d2}_{ci}") for d2 in range(D2)]
                for d2 in range(D2):
                    nc.sync.dma_start(
                        out=tiles[d2].rearrange("p (i c) -> p i c", i=2),
                        in_=vT[d2 * 256:(d2 + 1) * 256, c0:c0 + cn].rearrange(
                            "(i p) c -> p i c", p=128))
                vx_t.append(tiles)
            for jp in range(D2):
                nc.gpsimd.dma_start(
                    out=wfc8_sb[jp].rearrange("p (i n) -> p i n", i=2),
                    in_=pair_ap(Wfc, jp))


        # ================= attention ====================================
        with tc.tile_pool(name="scps", bufs=2, space="PSUM") as psc, \
             tc.tile_pool(name="cxps", bufs=1, space="PSUM") as pcx, \
             tc.tile_pool(name="pjps", bufs=2, space="PSUM") as ppj, \
             tc.tile_pool(name="qpT", bufs=2) as pqpt, \
             tc.tile_pool(name="attn", bufs=8) as patn, \
             tc.tile_pool(name="norm", bufs=1) as pnm, \
             tc.tile_pool(name="cpair", bufs=1) as pcp, \
             tc.tile_pool(name="ctmp", bufs=2) as ptmp:
            qpT_tiles = {}
            vp_ready = [0]
            kp_ready = [0] * EB  # chunks of kpT[e] projected

            # ---- K-projection thunk groups (plain fp8, per (e, ci)) ---
            def make_k_groups():
                groups = []

                def k_mm(e, ci, cn, d2, state):
                    def f():
                        if 'ps' not in state:
                            state['ps'] = ppj.tile([128, CK], F32, tag="pj",
                                                   name="pjk")
                        wv_ = wk8_sb[d2].rearrange("p (i n) -> p i n", i=2)
                        kxv = kx_t[ci][d2].rearrange("p (i c) -> p i c", i=2)
                        nc.tensor.matmul(
                            state['ps'][:, 0:cn],
                            lhsT=wv_[:, :, e * 128:(e + 1) * 128],
                            rhs=kxv[:, :, 0:cn],
                            start=(d2 == 0), stop=(d2 == D2 - 1),
                            perf_mode=DR)
                    return f

                def k_evac(e, ci, c0, cn, state):
                    def f():
                        nc.vector.tensor_scalar(
                            out=kpT_sb[e][:, c0:c0 + cn],
                            in0=state['ps'][:, 0:cn],
                            scalar1=1.0 / 64, scalar2=bkT[:, e:e + 1],
                            op0=ALU.mult, op1=ALU.add)
                        kp_ready[e] += 1
                    return f

                for e in range(EB):
                    for ci, (c0, cn) in enumerate(c0s):
                        state = {}
                        g = [k_mm(e, ci, cn, d2, state)
                             for d2 in range(D2)]
                        g.append(k_evac(e, ci, c0, cn, state))
                        groups.append(g)
                return groups

            # ---- V-projection thunks (fp8 DoubleRow, n-half groups) ---
            def make_v_thunks():
                thunks = []

                def v_mm(s, half, n0, nn, d2, state):
                    def f():
                        if half not in state:
                            state[half] = ppj.tile([128, 512], F32, tag="pj",
                                                   name="pjv")
                            nc.tensor.matmul(
                                state[half][:, 0:nn], lhsT=ones_1,
                                rhs=bv64_n[0:1, n0:n0 + nn],
                                start=True, stop=False)
                        vxv = vx_t[s // SPC][d2].rearrange(
                            "p (i c) -> p i c", i=2)
                        wvv = wv8_sb[d2].rearrange("p (i n) -> p i n", i=2)
                        nc.tensor.matmul(
                            state[half][:, 0:nn],
                            lhsT=vxv[:, :, (s % SPC) * 128:(s % SPC + 1) * 128],
                            rhs=wvv[:, :, n0:n0 + nn],
                            start=False, stop=(d2 == D2 - 1),
                            perf_mode=DR)
                    return f

                def v_evac(s, half, n0, nn, state):
                    def f():
                        # features n0:n0+nn == heads n0//64 : (n0+nn)//64
                        h0, h1 = n0 // DK, (n0 + nn) // DK
                        vr = vp2_sb[s // 2].rearrange(
                            "p (i h c) -> p i h c", i=2, c=VW)
                        nc.vector.tensor_scalar(
                            out=vr[:, s % 2, h0:h1, 0:DK],
                            in0=state[half].rearrange(
                                "p (h c) -> p h c", c=DK)[:, 0:(h1 - h0), :],
                            scalar1=0.125, scalar2=None, op0=ALU.mult)
                        nc.vector.memset(vr[:, s % 2, h0:h1, DK:VW], 8.0)
                        if half == 1 and s % 2 == 1:
                            vp_ready[0] += 1
                    return f

                for s in range(SB):
                    state = {}
                    for half, (n0, nn) in enumerate(nsplits(D)):
                        g = [v_mm(s, half, n0, nn, d2, state)
                             for d2 in range(D2)]
                        g.append(v_evac(s, half, n0, nn, state))
                        thunks.append(g)
                return thunks

            # ---- Q^T projection (bf16) + residual qp (x4096) thunks ---
            def make_q_thunks(jj):
                state = {}
                thunks = []

                def q_mm(half, n0, nn, d):
                    def f():
                        if half not in state:
                            state[half] = ppj.tile([128, 512], F32, tag="pj",
                                                   name="pjq")
                        nc.tensor.matmul(
                            state[half][:, 0:nn],
                            lhsT=wq_sb[d][:, jj * 128:(jj + 1) * 128],
                            rhs=qx_sb[d][:, n0:n0 + nn],
                            start=(d == 0), stop=(d == DB - 1))
                    return f

                def q_evac(half, n0, nn):
                    def f():
                        if 'qt' not in state:
                            state['qt'] = pqpt.tile([128, T], BF16,
                                                    tag="qpT_t", name="qpT_t")
                            qpT_tiles[jj] = state['qt']
                        nc.vector.tensor_scalar(
                            out=state['qt'][:, n0:n0 + nn],
                            in0=state[half][:, 0:nn],
                            scalar1=bqT[:, jj:jj + 1], scalar2=None,
                            op0=ALU.add)
                    return f

                for half, (n0, nn) in enumerate(nsplits(T)):
                    for d in range(DB):
                        thunks.append(q_mm(half, n0, nn, d))
                    thunks.append(q_evac(half, n0, nn))
                return thunks

            def make_qp_thunks(t):
                """Residual qp[t] = 4096*(q @ Wq + bq) per 128-token block,
                bf16; the 4096 matches the fp8 FC scale, layernorm is
                scale-invariant."""
                state = {}
                thunks = []

                def qp_mm(half, n0, nn, d):
                    def f():
                        if half not in state:
                            state[half] = ppj.tile([128, 512], F32, tag="pj",
                                                   name="pjr")
                        nc.tensor.matmul(
                            state[half][:, 0:nn],
                            lhsT=qx_sb[d][:, t * 128:(t + 1) * 128],
                            rhs=wq_sb[d][:, n0:n0 + nn],
                            start=(d == 0), stop=(d == DB - 1))
                    return f

                def qp_evac(half, n0, nn):
                    def f():
                        nc.vector.tensor_scalar(
                            out=qp_sb[t][:, n0:n0 + nn],
                            in0=state[half][:, 0:nn],
                            scalar1=4096.0, scalar2=None, op0=ALU.mult)
                    return f

                for half, (n0, nn) in enumerate(nsplits(D)):
                    for d in range(DB):
                        thunks.append(qp_mm(half, n0, nn, d))
                    thunks.append(qp_evac(half, n0, nn))
                return thunks

            kg = make_k_groups()
            # eager: kpT[0] chunk 0, then pair 0's Q^T (so the two DMA
            # stalls overlap), then kpT[0] chunks 1..3
            for th in kg[0]:
                th()
            for th in make_q_thunks(0):
                th()
            for g in kg[1:NCK]:
                for th in g:
                    th()
            # mixed background queue: 1 K-group : 2 V-half-groups, so the
            # per-pair kpT gate leads the per-kb vp gate comfortably
            vg = make_v_thunks()
            vq = deque()
            ki, vi = NCK, 0
            while ki < len(kg) or vi < len(vg):
                if ki < len(kg):
                    vq.extend(kg[ki])
                    ki += 1
                for _ in range(2):
                    if vi < len(vg):
                        vq.extend(vg[vi])
                        vi += 1

            av_jobs = deque()

            def drain_av():
                while av_jobs and 2 * vp_ready[0] > av_jobs[0][1]:
                    h_, kb_, cx_, at_ = av_jobs.popleft()
                    vr = vp2_sb[kb_ // 2].rearrange("p (i h c) -> p i h c",
                                                    i=2, c=VW)
                    for n0, nn in nsplits(T):
                        nc.tensor.matmul(
                            cx_[:, n0:n0 + nn],
                            lhsT=vr[:, kb_ % 2, h_, :],
                            rhs=at_[:, n0:n0 + nn],
                            start=(kb_ == 0), stop=(kb_ == SB - 1))

            for j in range(PAIRS):
                # hard gate: kpT[j] must be fully projected before scores
                while kp_ready[j] < NCK:
                    vq.popleft()()
                qpT_t = qpT_tiles.pop(j)
                if debug and j == 0:
                    nc.sync.dma_start(out=dbg["dbg_qpT"], in_=qpT_t)
                pending = list(make_q_thunks(j + 1)) if j + 1 < PAIRS else []
                if j < PAIRS - 1:
                    pending += make_qp_thunks(j)
                    if j == PAIRS - 2:
                        pending += make_qp_thunks(PAIRS - 1)
                else:
                    pending += make_qp_thunks(PAIRS - 1)
                dbc = pnm.tile([128, T], F32, tag="dbc", name="dbc")
                for hh in range(2):
                    h = 2 * j + hh
                    pr = slice(hh * 64, hh * 64 + 64)
                    cx = pcx.tile([VW, T], F32, tag="cx", name="cx")
                    for kb in range(SB):
                        at = patn.tile([128, T], BF16, tag="at", name="at")
                        sc = psc.tile([128, T], F32, tag="sc", name="sc")
                        for n0, nn in nsplits(T):
                            nc.tensor.matmul(
                                sc[:, n0:n0 + nn],
                                lhsT=kpT_sb[j][pr, kb * 128:(kb + 1) * 128],
                                rhs=qpT_t[pr, n0:n0 + nn],
                                start=True, stop=True)
                        nc.scalar.activation(out=at, in_=sc,
                                             func=AF.Exp, scale=0.125)
                        for _ in range(6 if j < 2 else 3):
                            if vq:
                                vq.popleft()()
                            elif pending:
                                pending.pop(0)()
                            else:
                                break
                        av_jobs.append((h, kb, cx, at))
                        drain_av()
                    # force-finish this head's attn@V before evacuating
                    while av_jobs:
                        while vq and 2 * vp_ready[0] <= av_jobs[0][1]:
                            vq.popleft()()
                        drain_av()
                    # evacuate unnormalized ctx (bf16) + denominator/64
                    den = pnm.tile([VW, T], F32, tag="den", name="den")
                    nc.vector.tensor_scalar(
                        out=den[DK:VW, :], in0=cx[DK:VW, :],
                        scalar1=1.0 / 64, scalar2=None, op0=ALU.mult)
                    nc.gpsimd.dma_start(out=den_dram[h, :], in_=den[DK:VW, :])
                    nc.gpsimd.dma_start(
                        out=dbc[hh * 64:(hh + 1) * 64, :],
                        in_=bcast_ap(den_dram[h:h + 1, :], 64))
                    if hh == 0:
                        cpair = pcp.tile([128, T], BF16, tag="cpair",
                                         name="cpair")
                        nc.vector.tensor_copy(out=cpair[0:64, :],
                                              in_=cx[0:DK, :])
                    else:
                        tmp = ptmp.tile([64, T], BF16, tag="ctmp", name="ctmp")
                        nc.vector.tensor_copy(out=tmp, in_=cx[0:DK, :])
                        nc.sync.dma_start(out=cpair[64:128, :], in_=tmp)
                while pending:
                    pending.pop(0)()
                # normalize: ctx8[jp][:, j%2, :] = cpair * (64/den)  in fp8
                rbc = pnm.tile([128, T], F32, tag="rbc", name="rbc")
                nc.vector.reciprocal_approx_fast(out=rbc, in_=dbc)
                c8 = ctx8_sb[j // 2].rearrange("p (i n) -> p i n", i=2)
                nc.vector.tensor_mul(out=c8[:, j % 2, :], in0=cpair, in1=rbc)

        # ========= FC (fp8 DoubleRow) + residual + layernorm ===========
        with tc.tile_pool(name="fcps", bufs=3, space="PSUM") as pfc, \
             tc.tile_pool(name="xln", bufs=3) as px, \
             tc.tile_pool(name="stat", bufs=6) as pst:
            for t in range(TB):
                fc = pfc.tile([128, D], F32, tag="fc", name="fc")
                for n0, nn in nsplits(D):
                    nc.tensor.matmul(
                        fc[:, n0:n0 + nn], lhsT=ones_1,
                        rhs=bqfc4096_n[0:1, n0:n0 + nn],
                        start=True, stop=False)
                    for jp in range(D2):
                        c8 = ctx8_sb[jp].rearrange("p (i n) -> p i n", i=2)
                        w8 = wfc8_sb[jp].rearrange("p (i n) -> p i n", i=2)
                        for i in range(2):
                            nc.tensor.matmul(
                                fc[:, n0:n0 + nn],
                                lhsT=c8[:, i, t * 128:(t + 1) * 128],
                                rhs=w8[:, i, n0:n0 + nn],
                                start=False,
                                stop=(jp == D2 - 1 and i == 1))
                x = px.tile([128, D], F32, tag="x", name="x")
                nc.vector.tensor_add(out=x, in0=fc, in1=qp_sb[t])
                ngr = max(D // 512, 1)
                gsz = min(D, 512)
                stats = pst.tile([128, ngr, 6], F32, tag="stats", name="stats")
                for g in range(ngr):
                    nc.vector.bn_stats(out=stats[:, g, :],
                                       in_=x[:, g * gsz:(g + 1) * gsz])
                mv = pst.tile([128, 2], F32, tag="mv", name="mv")
                nc.vector.bn_aggr(out=mv, in_=stats)
                rstd = pst.tile([128, 1], F32, tag="rstd", name="rstd")
                nc.scalar.activation(out=rstd, in_=mv[:, 1:2], func=AF.Sqrt,
                                     bias=eps_t, scale=1.0)
                nc.vector.reciprocal(out=rstd, in_=rstd)
                xn = px.tile([128, D], F32, tag="xn", name="xn")
                nc.vector.tensor_scalar(out=xn, in0=x, scalar1=mv[:, 0:1],
                                        scalar2=rstd, op0=ALU.subtract,
                                        op1=ALU.mult)
                nc.vector.tensor_mul(out=xn, in0=xn, in1=gamma_bc)
                nc.gpsimd.tensor_add(out=xn, in0=xn, in1=beta_bc)
                nc.sync.dma_start(out=out[t * 128:(t + 1) * 128, :], in_=xn)

            if debug:
                nc.sync.dma_start(out=dbg["dbg_kpT"], in_=kpT_sb[0])
                nc.sync.dma_start(out=dbg["dbg_vp"], in_=vp2_sb[0])
                nc.sync.dma_start(out=dbg["dbg_ctx8"], in_=ctx8_sb[0])
                nc.sync.dma_start(out=dbg["dbg_qp"], in_=qp_sb[0])

    nc.compile()
    return nc


_B, _S, _D, _H, _DK = 4, 2048, 1024, 16, 64
_T = _S // 2
_NCORES = 8
_BF = ml_dtypes.bfloat16
_F8 = ml_dtypes.float8_e4m3

_nc_cache = [None]


def _get_nc():
    if _nc_cache[0] is None:
        _nc_cache[0] = build(T=_T, S=_S, D=_D, H=_H, DK=_DK, n_cores=_NCORES)
    return _nc_cache[0]


def _f8(a):
    return np.clip(a, -240.0, 240.0).astype(_F8)


def _make_in_maps(inputs):
    q = np.asarray(inputs["q"], np.float32)
    k = np.asarray(inputs["k"], np.float32)
    v = np.asarray(inputs["v"], np.float32)
    Wq = np.asarray(inputs["Wq"], np.float32).astype(_BF)
    Wk = _f8(np.asarray(inputs["Wk"], np.float32) * 64.0)
    Wv = _f8(np.asarray(inputs["Wv"], np.float32) * 64.0)
    Wfc = _f8(np.asarray(inputs["Wfc"], np.float32) * 64.0)
    fp = {n: np.asarray(inputs[n], np.float32)
          for n in ("bq", "bk", "bv", "bfc", "gamma", "beta")}

    in_maps = []
    for c in range(_NCORES):
        b, half = divmod(c, 2)
        t0 = half * _T
        in_maps.append({
            "qT": np.ascontiguousarray(q[b, t0:t0 + _T].T).astype(_BF),
            "kT": _f8(np.ascontiguousarray(k[b].T)),
            "vT": _f8(np.ascontiguousarray(v[b].T)),
            "Wq": Wq, "Wk": Wk, "Wv": Wv, "Wfc": Wfc, **fp,
        })
    return in_maps


def _execute(inputs, trace=False, tmpdir=None):
    from concourse.bass_utils import run_bass_kernel_spmd

    nc = _get_nc()
    in_maps = _make_in_maps(inputs)
    res = run_bass_kernel_spmd(nc, in_maps, core_ids=list(range(_NCORES)),
                               trace=trace, tmpdir=tmpdir)
    out = np.empty((_B, _S, _D), np.float32)
    for c in range(_NCORES):
        b, half = divmod(c, 2)
        out[b, half * _T:(half + 1) * _T] = res.results[c]["out"]
    return out, res.exec_time_ns


def kernel(**inputs) -> np.ndarray:
    out, _ = _execute(inputs, trace=False)
    return out
